# revision 17
# baseline (speedup 1.0000x reference)
"""Multi-head self-attention TRN2 Bass kernel (v2).

Sharding: 8 cores = 4 batches x 2 head-halves. Core (b, half) computes
batch b with 8 heads (half*8 .. half*8+8), producing a [CAP, 1024]
partial of the output projection; the host sums the 2 halves per batch,
scatters compacted rows back to their original positions, and adds the
constant bv @ Wc.T row (softmax weights sum to 1, so the V-bias
contributes a constant vector).

Host-side compaction: only unmasked tokens (mask==0) take part in
attention -- masked queries produce zero rows and masked keys are
excluded. Tokens are compacted per batch and padded to CAP=1152
(valid counts are ~1004-1036), cutting attention work to (1152/2048)^2
and projection work to 1152/2048.

All matmuls run in fp16 (1 cycle/row on the PE at full clock, fast
weight loads); fp32r was 2-4x slower per row because it self-loads
weights and kept the PE at the cold 1.2 GHz p-state. Dots for the two
heads of a pair run concurrently via PE row-tiling (tile_position
(0,0)/(64,0), contraction 64 each).

Padding keys are killed with a per-partition bias on the exp
activation: p = exp(dots/8 - 1) for valid keys, exp(dots/8 - 31) ~ 0
for padded keys. The diagonal (self-attention) is zeroed by a DVE
multiply with (1-I) on the 128-column window that contains it. The
softmax denominator comes from a 65th all-ones column in the V'
stationary. Normalization: rep = broadcast(1/den) via a tiny
ones-stationary matmul, then one DVE multiply that also moves
O from PSUM to SBUF.
"""

import os
import numpy as np
import ml_dtypes

import concourse.bacc as bacc
import concourse.mybir as mybir
from concourse.tile import TileContext
from concourse.bass_utils import run_bass_kernel_spmd

B, S, H, NH, HD = 4, 2048, 1024, 16, 64
NCORES = 8
HPC = 8                   # heads per core
PD = HPC * HD             # per-core projection dim = 512
CAP = 1152                # compacted token capacity per batch
KT = CAP // 128           # 9 key tiles
FT = H // 128             # 8 feature k-tiles
QCH = [(0, 512), (512, 512), (1024, 128)]   # query chunks
F32 = mybir.dt.float32
F16 = mybir.dt.float16

LAST_RESULTS = None       # BassKernelResults from the most recent run


def build_bass(cap=CAP, fused_pairs=3):
    """fused_pairs: number of leading kt-pairs whose keys are known
    all-valid (bias -1 immediate, one exp instr per 2 key tiles).
    The remaining kt get per-kt exp instrs with a per-partition bias AP
    that encodes padding. fused_pairs=3 requires nb >= 896."""
    kt = cap // 128
    qch = [(o, n) for (o, n) in [(i * 512, min(512, cap - i * 512))
                                 for i in range((cap + 511) // 512)] if n > 0]
    nc = bacc.Bacc()
    xcT = nc.dram_tensor("xcT", [H, cap], F16, kind="ExternalInput")
    wq = nc.dram_tensor("wq", [H, PD], F16, kind="ExternalInput")
    wk = nc.dram_tensor("wk", [H, PD], F16, kind="ExternalInput")
    wv = nc.dram_tensor("wv", [H, PD], F16, kind="ExternalInput")
    wc = nc.dram_tensor("wc", [PD, H], F16, kind="ExternalInput")
    bqk = nc.dram_tensor("bqk", [128, 8], F32, kind="ExternalInput")
    padb = nc.dram_tensor("padb", [128, kt], F32, kind="ExternalInput")
    eyebar = nc.dram_tensor("eyebar", [128, 128], F16, kind="ExternalInput")
    ones64 = nc.dram_tensor("ones64", [1, 64], F16, kind="ExternalInput")
    outp = nc.dram_tensor("out", [cap, H], F16, kind="ExternalOutput")

    EXP = mybir.ActivationFunctionType.Exp
    DR = None  # no DoubleRow (fp16 path)

    # exp schedule: list of (kts, bias_mode); bias_mode None -> -1.0 imm
    sched = []
    for i in range(fused_pairs):
        sched.append(((2 * i, 2 * i + 1), None))
    for k in range(2 * fused_pairs, kt):
        sched.append(((k,), k))
    # which kt are known all-valid without AP bias
    for (kts, bm) in sched:
        if bm is None:
            assert max(kts) < 2 * fused_pairs

    with TileContext(nc) as tc, \
         tc.tile_pool(name="consts", bufs=1) as cpool, \
         tc.tile_pool(name="work", bufs=1) as wpool, \
         tc.tile_pool(name="ptp", bufs=4) as ppool, \
         tc.tile_pool(name="small", bufs=3) as spool, \
         tc.tile_pool(name="osb", bufs=2) as opool, \
         tc.tile_pool(name="psum", bufs=1, space="PSUM") as pspool:

        # ---- constants / weights ----
        w_sb = {}
        for name, t in (("q", wq), ("k", wk), ("v", wv)):
            sb = cpool.tile([128, FT, PD], F16, name=f"w{name}sb")
            for ft in range(FT):
                nc.sync.dma_start(out=sb[:, ft, :],
                                  in_=t[ft * 128:(ft + 1) * 128, :])
            w_sb[name] = sb
        wc_sb = cpool.tile([128, 4, H], F16, name="wcsb")
        for g in range(4):
            nc.sync.dma_start(out=wc_sb[:, g, :],
                              in_=wc[g * 128:(g + 1) * 128, :])
        bqk_sb = cpool.tile([128, 8], F32, name="bqksb")
        nc.sync.dma_start(out=bqk_sb[:, :], in_=bqk[:, :])
        padb_sb = cpool.tile([128, kt], F32, name="padbsb")
        nc.sync.dma_start(out=padb_sb[:, :], in_=padb[:, :])
        eye_sb = cpool.tile([128, 128], F16, name="eyesb")
        nc.sync.dma_start(out=eye_sb[:, :], in_=eyebar[:, :])
        ones_sb = cpool.tile([1, 64], F16, name="onessb")
        nc.sync.dma_start(out=ones_sb[:, :], in_=ones64[:, :])

        xc_sb = wpool.tile([128, FT, cap], F16, name="xcsb")
        for ft in range(FT):
            nc.sync.dma_start(out=xc_sb[:, ft, :],
                              in_=xcT[ft * 128:(ft + 1) * 128, :])

        # ---- q/k projections -> qkT [128, 4, cap] f16 ----
        qkT = {w: wpool.tile([128, 4, cap], F16, name=f"{w}T")
               for w in "qk"}
        for wi, w in enumerate("qk"):
            for g in range(4):
                for qo, qn in qch:
                    pp = pspool.tile([128, 512], F32, tag="mm", bufs=2)
                    for ft in range(FT):
                        nc.tensor.matmul(
                            pp[:, 0:qn],
                            w_sb[w][:, ft, g * 128:(g + 1) * 128],
                            xc_sb[:, ft, qo:qo + qn],
                            start=(ft == 0), stop=(ft == FT - 1))
                    nc.vector.tensor_scalar_add(
                        qkT[w][:, g, qo:qo + qn], pp[:, 0:qn],
                        bqk_sb[:, 4 * wi + g:4 * wi + g + 1])

        # ---- v projection (token-major) -> vp [128, kt, 8, 65] f16 ----
        # vp[key, kt, h, m] = V[key, h*64+m]; vp[key, kt, h, 64] = 1 (denom)
        vp = wpool.tile([128, kt, HPC, 65], F16, name="vp")
        for tt in range(kt):
            pv = pspool.tile([128, 512], F32, tag="mm", bufs=2)
            for ft in range(FT):
                nc.tensor.matmul(
                    pv[:, :],
                    xc_sb[:, ft, tt * 128:(tt + 1) * 128],
                    w_sb["v"][:, ft, :],
                    start=(ft == 0), stop=(ft == FT - 1))
            # strided copy: [128, 8, 64] from pv into the 65-strided vp
            nc.vector.tensor_copy(
                vp[:, tt, :, 0:64],
                pv.rearrange("p (h d) -> p h d", d=64))
        nc.vector.memset(vp[:, :, :, 64:65], 1.0)

        # ---- attention, 3-stage software pipeline over (pair, qchunk) ----
        # stage 1: dots + exp + diag -> ptp tiles
        # stage 2 (one step behind): AV accumulation + reciprocal issue
        # stage 3 (two steps behind): rep broadcast + normalize
        # This keeps the PE fed: exp(i) overlaps AV(i-1) on the PE, and the
        # slow [1,N] reciprocal gets a full step of slack before rep reads it.
        onorm = wpool.tile([128, 4, cap], F16, name="onorm")
        avq = []        # (ptp pair, p, qo, qn) awaiting AV
        deferred = []   # (avs, rc, p, rb, qo, qn) awaiting rep+normalize

        def flush_norm():
            while deferred:
                avs_d, rc_d, p_d, rb_d, qo_d, qn_d = deferred.pop(0)
                rep = pspool.tile([64, 512], F32, tag="mm", bufs=2,
                                  name="rep")
                nc.tensor.matmul(rep[:, 0:qn_d], ones_sb[:, :],
                                 rc_d[:, 0:qn_d], start=True, stop=True)
                rep_sb = spool.tile([64, 512], F16, tag="repsb",
                                    name="repsb")
                nc.vector.tensor_copy(rep_sb[:, 0:qn_d], rep[:, 0:qn_d])
                nc.vector.tensor_mul(
                    onorm[rb_d:rb_d + 64, p_d, qo_d:qo_d + qn_d],
                    avs_d[0:64, 0:qn_d], rep_sb[:, 0:qn_d])

        def do_av(ptp_a, p_a, qo_a, qn_a):
            if len(deferred) >= 2:
                flush_norm()
            for h01 in range(2):
                h = 2 * p_a + h01
                rb = h01 * 64
                avs = pspool.tile([65, 512], F32, tag="avs", bufs=2,
                                  name="avs")
                for k in range(kt):
                    nc.tensor.matmul(
                        avs[:, 0:qn_a],
                        vp[:, k, h, :],
                        ptp_a[h01][:, k, 0:qn_a],
                        start=(k == 0), stop=(k == kt - 1),
                        perf_mode=DR)
                rc = spool.tile([1, 512], F16, tag="rc", name="rc")
                with nc.allow_low_precision(
                        reason="1/den in fp16: 0.05% rel, den>=13"):
                    nc.vector.reciprocal(rc[:, 0:qn_a], avs[64:65, 0:qn_a])
                deferred.append((avs, rc, p_a, rb, qo_a, qn_a))

        for p in range(4):
            for qo, qn in qch:
                # AV of the previous step first: its reciprocal lands in
                # the DVE queue ahead of this step's diag multiplies
                # (which wait on the exp tail)
                if avq:
                    do_av(*avq.pop(0))
                ptp = [ppool.tile([128, kt, 512], F16, tag=f"ptp{h01}",
                                  bufs=2, name=f"ptp{p}{h01}{qo}")
                       for h01 in range(2)]
                for h01 in range(2):
                    rb = h01 * 64
                    for kts, bm in sched:
                        dp = pspool.tile([128, 2, 512], F32, tag="dp", bufs=2)
                        for j, k in enumerate(kts):
                            nc.tensor.matmul(
                                dp[:, j, 0:qn],
                                qkT["k"][rb:rb + 64, p,
                                         k * 128:(k + 1) * 128],
                                qkT["q"][rb:rb + 64, p, qo:qo + qn],
                                start=True, stop=True,
                                tile_position=(rb, 0))
                        # fused groups: all keys valid -> their padb col
                        # is uniformly -1, so kts[0]'s column works
                        bias = padb_sb[:, kts[0]:kts[0] + 1] if bm is None \
                            else padb_sb[:, bm:bm + 1]
                        nc.scalar.activation(
                            ptp[h01][:, kts[0]:kts[0] + len(kts), 0:qn],
                            dp[:, 0:len(kts), 0:qn],
                            EXP, scale=0.125, bias=bias)
                    # diagonal zeroing for kt windows inside this q chunk
                    k0, k1 = qo // 128, (qo + qn) // 128
                    for k in range(k0, k1):
                        off = k * 128 - qo
                        nc.vector.tensor_mul(
                            ptp[h01][:, k, off:off + 128],
                            ptp[h01][:, k, off:off + 128],
                            eye_sb[:, :])
                avq.append((ptp, p, qo, qn))
        while avq:
            do_av(*avq.pop(0))
        flush_norm()

        # ---- output projection ----
        for tt in range(kt):
            osb = opool.tile([128, H], F16, tag="osb")
            for oc in range(2):
                op = pspool.tile([128, 512], F32, tag="mm", bufs=2)
                for g in range(4):
                    nc.tensor.matmul(
                        op[:, :],
                        onorm[:, g, tt * 128:(tt + 1) * 128],
                        wc_sb[:, g, oc * 512:(oc + 1) * 512],
                        start=(g == 0), stop=(g == 3))
                nc.vector.tensor_copy(osb[:, oc * 512:(oc + 1) * 512],
                                      op[:, :])
            nc.sync.dma_start(
                out=outp[tt * 128:(tt + 1) * 128, :], in_=osb[:, :])
    nc.finalize()
    return nc


_NC_CACHE = {}


def _get_nc(cap, fused_pairs):
    key = (cap, fused_pairs)
    if key not in _NC_CACHE:
        _NC_CACHE[key] = build_bass(cap, fused_pairs)
    return _NC_CACHE[key]


def kernel(encoder_outputs, mask, Wq, bq, Wk, bk, Wv, bv, Wc):
    global LAST_RESULTS
    x = np.asarray(encoder_outputs, dtype=np.float32)
    mask = np.asarray(mask)
    f16 = np.float16
    Wqh, Wkh, Wvh = (np.asarray(w, np.float32) for w in (Wq, Wk, Wv))
    Wch = np.asarray(Wc, np.float32)

    idxs = [np.where(mask[b] == 0)[0] for b in range(B)]
    nbs = [len(i) for i in idxs]
    if max(nbs) <= CAP and min(nbs) >= 896:
        cap, fused = CAP, 3
    else:
        cap, fused = S, 0          # generic fallback: no compaction gain
    kt = cap // 128

    eyebar = (1.0 - np.eye(128, dtype=np.float32)).astype(f16)
    onesv = np.ones((1, 64), dtype=f16)

    in_maps = []
    for c in range(NCORES):
        b, half = c // 2, c % 2
        hsl = slice(half * PD, (half + 1) * PD)
        idx, nb = idxs[b], nbs[b]
        xc = np.zeros((cap, H), np.float32)
        xc[:nb] = x[b, idx[:nb]]
        xcT = np.ascontiguousarray(xc.T).astype(f16)
        padb = np.where(np.arange(cap) < nb, -1.0, -31.0).astype(np.float32)
        bqk = np.stack([np.asarray(bq, np.float32)[hsl].reshape(4, 128),
                        np.asarray(bk, np.float32)[hsl].reshape(4, 128)]
                       ).reshape(8, 128).T.copy()
        in_maps.append({
            "xcT": xcT,
            "wq": np.ascontiguousarray(Wqh[hsl, :].T).astype(f16),
            "wk": np.ascontiguousarray(Wkh[hsl, :].T).astype(f16),
            "wv": np.ascontiguousarray(Wvh[hsl, :].T).astype(f16),
            "wc": np.ascontiguousarray(Wch[:, hsl].T).astype(f16),
            "bqk": bqk,
            "padb": np.ascontiguousarray(padb.reshape(kt, 128).T),
            "eyebar": eyebar,
            "ones64": onesv,
        })

    res = run_bass_kernel_spmd(
        _get_nc(cap, fused), in_maps, list(range(NCORES)),
        trace=bool(os.environ.get("BASS_TRACE")))
    LAST_RESULTS = res

    bvWc = (np.asarray(bv, np.float64) @ np.asarray(Wc, np.float64).T
            ).astype(np.float32)
    out = np.zeros((B, S, H), dtype=np.float32)
    for b in range(B):
        nb = nbs[b]
        part = (res.results[2 * b]["out"][:nb].astype(np.float32)
                + res.results[2 * b + 1]["out"][:nb].astype(np.float32))
        out[b, idxs[b][:nb]] = part + bvWc[None, :]
    return out


# revision 20
# speedup vs baseline: 1.1787x; 1.1787x over previous
"""Multi-head self-attention TRN2 Bass kernel (v2).

Sharding: 8 cores = 4 batches x 2 head-halves. Core (b, half) computes
batch b with 8 heads (half*8 .. half*8+8), producing a [CAP, 1024]
partial of the output projection; the host sums the 2 halves per batch,
scatters compacted rows back to their original positions, and adds the
constant bv @ Wc.T row (softmax weights sum to 1, so the V-bias
contributes a constant vector).

Host-side compaction: only unmasked tokens (mask==0) take part in
attention -- masked queries produce zero rows and masked keys are
excluded. Tokens are compacted per batch and padded to CAP=1152
(valid counts are ~1004-1036), cutting attention work to (1152/2048)^2
and projection work to 1152/2048.

All matmuls run in fp16 (1 cycle/row on the PE at full clock, fast
weight loads); fp32r was 2-4x slower per row because it self-loads
weights and kept the PE at the cold 1.2 GHz p-state. Dots for the two
heads of a pair run concurrently via PE row-tiling (tile_position
(0,0)/(64,0), contraction 64 each).

Padding keys are killed with a per-partition bias on the exp
activation: p = exp(dots/8 - 1) for valid keys, exp(dots/8 - 31) ~ 0
for padded keys. The diagonal (self-attention) is zeroed by a DVE
multiply with (1-I) on the 128-column window that contains it. The
softmax denominator comes from a 65th all-ones column in the V'
stationary. Normalization: rep = broadcast(1/den) via a tiny
ones-stationary matmul, then one DVE multiply that also moves
O from PSUM to SBUF.
"""

import os
import numpy as np
import ml_dtypes

import concourse.bacc as bacc
import concourse.mybir as mybir
from concourse.tile import TileContext
from concourse.bass_utils import run_bass_kernel_spmd

B, S, H, NH, HD = 4, 2048, 1024, 16, 64
NCORES = 8
HPC = 8                   # heads per core
PD = HPC * HD             # per-core projection dim = 512
CAP = 1152                # compacted token capacity per batch
KT = CAP // 128           # 9 key tiles
FT = H // 128             # 8 feature k-tiles
QCH = [(0, 512), (512, 512), (1024, 128)]   # query chunks
F32 = mybir.dt.float32
F16 = mybir.dt.float16

LAST_RESULTS = None       # BassKernelResults from the most recent run


def build_bass(cap=CAP, fused_pairs=3):
    """fused_pairs: number of leading kt-pairs whose keys are known
    all-valid (bias -1 immediate, one exp instr per 2 key tiles).
    The remaining kt get per-kt exp instrs with a per-partition bias AP
    that encodes padding. fused_pairs=3 requires nb >= 896."""
    kt = cap // 128
    qch = [(o, n) for (o, n) in [(i * 512, min(512, cap - i * 512))
                                 for i in range((cap + 511) // 512)] if n > 0]
    if cap == CAP:
        # attention stages only need query columns < nb (max valid count
        # is ~1036); trim the last chunk 128 -> 64 (covers nb <= 1088,
        # checked at dispatch). Projections/outproj still cover all of cap.
        qch[-1] = (qch[-1][0], 64)
    nc = bacc.Bacc()
    xcT = nc.dram_tensor("xcT", [H, cap], F16, kind="ExternalInput")
    wq = nc.dram_tensor("wq", [H, PD], F16, kind="ExternalInput")
    wk = nc.dram_tensor("wk", [H, PD], F16, kind="ExternalInput")
    wv = nc.dram_tensor("wv", [H, PD], F16, kind="ExternalInput")
    wc = nc.dram_tensor("wc", [PD, H], F16, kind="ExternalInput")
    bqk = nc.dram_tensor("bqk", [128, 8], F32, kind="ExternalInput")
    padb = nc.dram_tensor("padb", [128, kt], F32, kind="ExternalInput")
    eyebar = nc.dram_tensor("eyebar", [128, 128], F16, kind="ExternalInput")
    ones64 = nc.dram_tensor("ones64", [1, 64], F16, kind="ExternalInput")
    outp = nc.dram_tensor("out", [cap, H], F16, kind="ExternalOutput")

    EXP = mybir.ActivationFunctionType.Exp
    DR = None  # no DoubleRow (fp16 path)

    # exp schedule: list of (kts, bias_mode); bias_mode None -> -1.0 imm
    sched = []
    for i in range(fused_pairs):
        sched.append(((2 * i, 2 * i + 1), None))
    for k in range(2 * fused_pairs, kt):
        sched.append(((k,), k))
    # which kt are known all-valid without AP bias
    for (kts, bm) in sched:
        if bm is None:
            assert max(kts) < 2 * fused_pairs

    with TileContext(nc) as tc, \
         tc.tile_pool(name="consts", bufs=1) as cpool, \
         tc.tile_pool(name="work", bufs=1) as wpool, \
         tc.tile_pool(name="ptp", bufs=4) as ppool, \
         tc.tile_pool(name="small", bufs=3) as spool, \
         tc.tile_pool(name="osb", bufs=2) as opool, \
         tc.tile_pool(name="psum", bufs=1, space="PSUM") as pspool:

        # ---- constants / weights ----
        w_sb = {}
        for name, t in (("q", wq), ("k", wk), ("v", wv)):
            sb = cpool.tile([128, FT, PD], F16, name=f"w{name}sb")
            for ft in range(FT):
                nc.sync.dma_start(out=sb[:, ft, :],
                                  in_=t[ft * 128:(ft + 1) * 128, :])
            w_sb[name] = sb
        wc_sb = cpool.tile([128, 4, H], F16, name="wcsb")
        for g in range(4):
            nc.sync.dma_start(out=wc_sb[:, g, :],
                              in_=wc[g * 128:(g + 1) * 128, :])
        bqk_sb = cpool.tile([128, 8], F32, name="bqksb")
        nc.sync.dma_start(out=bqk_sb[:, :], in_=bqk[:, :])
        padb_sb = cpool.tile([128, kt], F32, name="padbsb")
        nc.sync.dma_start(out=padb_sb[:, :], in_=padb[:, :])
        eye_sb = cpool.tile([128, 128], F16, name="eyesb")
        nc.sync.dma_start(out=eye_sb[:, :], in_=eyebar[:, :])
        ones_sb = cpool.tile([1, 64], F16, name="onessb")
        nc.sync.dma_start(out=ones_sb[:, :], in_=ones64[:, :])

        xc_sb = wpool.tile([128, FT, cap], F16, name="xcsb")
        for ft in range(FT):
            nc.sync.dma_start(out=xc_sb[:, ft, :],
                              in_=xcT[ft * 128:(ft + 1) * 128, :])

        # ---- q/k projections -> qkT [128, 4, cap] f16 ----
        qkT = {w: wpool.tile([128, 4, cap], F16, name=f"{w}T")
               for w in "qk"}
        for wi, w in enumerate("qk"):
            for g in range(4):
                for qo, qn in qch:
                    pp = pspool.tile([128, 512], F32, tag="mm", bufs=2)
                    for ft in range(FT):
                        nc.tensor.matmul(
                            pp[:, 0:qn],
                            w_sb[w][:, ft, g * 128:(g + 1) * 128],
                            xc_sb[:, ft, qo:qo + qn],
                            start=(ft == 0), stop=(ft == FT - 1))
                    nc.vector.tensor_scalar_add(
                        qkT[w][:, g, qo:qo + qn], pp[:, 0:qn],
                        bqk_sb[:, 4 * wi + g:4 * wi + g + 1])

        # ---- v projection (token-major) -> vp [128, kt, 8, 65] f16 ----
        # vp[key, kt, h, m] = V[key, h*64+m]; vp[key, kt, h, 64] = 1 (denom)
        vp = wpool.tile([128, kt, HPC, 65], F16, name="vp")
        for tt in range(kt):
            pv = pspool.tile([128, 512], F32, tag="mm", bufs=2)
            for ft in range(FT):
                nc.tensor.matmul(
                    pv[:, :],
                    xc_sb[:, ft, tt * 128:(tt + 1) * 128],
                    w_sb["v"][:, ft, :],
                    start=(ft == 0), stop=(ft == FT - 1))
            # strided copy: [128, 8, 64] from pv into the 65-strided vp
            nc.vector.tensor_copy(
                vp[:, tt, :, 0:64],
                pv.rearrange("p (h d) -> p h d", d=64))
        nc.vector.memset(vp[:, :, :, 64:65], 1.0)

        # ---- attention, 3-stage software pipeline over (pair, qchunk) ----
        # stage 1: dots + exp + diag -> ptp tiles
        # stage 2 (one step behind): AV accumulation + reciprocal issue
        # stage 3 (two steps behind): rep broadcast + normalize
        # This keeps the PE fed: exp(i) overlaps AV(i-1) on the PE, and the
        # slow [1,N] reciprocal gets a full step of slack before rep reads it.
        onorm = wpool.tile([128, 4, cap], F16, name="onorm")
        avq = []        # (ptp pair, p, qo, qn) awaiting AV
        deferred = []   # (avs, rc, p, rb, qo, qn) awaiting rep+normalize

        def flush_norm():
            while deferred:
                avs_d, rc_d, p_d, rb_d, qo_d, qn_d = deferred.pop(0)
                rep = pspool.tile([64, 512], F32, tag="mm", bufs=2,
                                  name="rep")
                nc.tensor.matmul(rep[:, 0:qn_d], ones_sb[:, :],
                                 rc_d[:, 0:qn_d], start=True, stop=True)
                rep_sb = spool.tile([64, 512], F16, tag="repsb",
                                    name="repsb")
                nc.vector.tensor_copy(rep_sb[:, 0:qn_d], rep[:, 0:qn_d])
                nc.vector.tensor_mul(
                    onorm[rb_d:rb_d + 64, p_d, qo_d:qo_d + qn_d],
                    avs_d[0:64, 0:qn_d], rep_sb[:, 0:qn_d])

        def do_av(ptp_a, p_a, qo_a, qn_a):
            if len(deferred) >= 2:
                flush_norm()
            for h01 in range(2):
                h = 2 * p_a + h01
                rb = h01 * 64
                avs = pspool.tile([65, 512], F32, tag="avs", bufs=2,
                                  name="avs")
                for k in range(kt):
                    nc.tensor.matmul(
                        avs[:, 0:qn_a],
                        vp[:, k, h, :],
                        ptp_a[h01][:, k, 0:qn_a],
                        start=(k == 0), stop=(k == kt - 1),
                        perf_mode=DR)
                den_sb = spool.tile([1, 512], F32, tag="densb", name="densb")
                nc.vector.tensor_copy(den_sb[:, 0:qn_a], avs[64:65, 0:qn_a])
                rc32 = spool.tile([1, 512], F32, tag="rc32", name="rc32")
                nc.vector.reciprocal_approx_fast(
                    rc32[:, 0:qn_a], den_sb[:, 0:qn_a])
                rc = spool.tile([1, 512], F16, tag="rc", name="rc")
                nc.vector.tensor_copy(rc[:, 0:qn_a], rc32[:, 0:qn_a])
                deferred.append((avs, rc, p_a, rb, qo_a, qn_a))

        for p in range(4):
            for qo, qn in qch:
                # AV of the previous step first: its reciprocal lands in
                # the DVE queue ahead of this step's diag multiplies
                # (which wait on the exp tail)
                if avq:
                    do_av(*avq.pop(0))
                ptp = [ppool.tile([128, kt, 512], F16, tag=f"ptp{h01}",
                                  bufs=2, name=f"ptp{p}{h01}{qo}")
                       for h01 in range(2)]
                for h01 in range(2):
                    rb = h01 * 64
                    for kts, bm in sched:
                        dp = pspool.tile([128, 2, 512], F32, tag="dp", bufs=2)
                        for j, k in enumerate(kts):
                            nc.tensor.matmul(
                                dp[:, j, 0:qn],
                                qkT["k"][rb:rb + 64, p,
                                         k * 128:(k + 1) * 128],
                                qkT["q"][rb:rb + 64, p, qo:qo + qn],
                                start=True, stop=True,
                                tile_position=(rb, 0))
                        # fused groups: all keys valid -> their padb col
                        # is uniformly -1, so kts[0]'s column works
                        bias = padb_sb[:, kts[0]:kts[0] + 1] if bm is None \
                            else padb_sb[:, bm:bm + 1]
                        nc.scalar.activation(
                            ptp[h01][:, kts[0]:kts[0] + len(kts), 0:qn],
                            dp[:, 0:len(kts), 0:qn],
                            EXP, scale=0.125, bias=bias)
                    # diagonal zeroing for kt windows inside this q chunk
                    k0, k1 = qo // 128, (qo + qn) // 128
                    for k in range(k0, k1):
                        off = k * 128 - qo
                        nc.vector.tensor_mul(
                            ptp[h01][:, k, off:off + 128],
                            ptp[h01][:, k, off:off + 128],
                            eye_sb[:, :])
                avq.append((ptp, p, qo, qn))
        while avq:
            do_av(*avq.pop(0))
        flush_norm()

        # ---- output projection ----
        for tt in range(kt):
            osb = opool.tile([128, H], F16, tag="osb")
            for oc in range(2):
                op = pspool.tile([128, 512], F32, tag="mm", bufs=2)
                for g in range(4):
                    nc.tensor.matmul(
                        op[:, :],
                        onorm[:, g, tt * 128:(tt + 1) * 128],
                        wc_sb[:, g, oc * 512:(oc + 1) * 512],
                        start=(g == 0), stop=(g == 3))
                nc.vector.tensor_copy(osb[:, oc * 512:(oc + 1) * 512],
                                      op[:, :])
            nc.sync.dma_start(
                out=outp[tt * 128:(tt + 1) * 128, :], in_=osb[:, :])
    nc.finalize()
    return nc


_NC_CACHE = {}


def _get_nc(cap, fused_pairs):
    key = (cap, fused_pairs)
    if key not in _NC_CACHE:
        _NC_CACHE[key] = build_bass(cap, fused_pairs)
    return _NC_CACHE[key]


def kernel(encoder_outputs, mask, Wq, bq, Wk, bk, Wv, bv, Wc):
    global LAST_RESULTS
    x = np.asarray(encoder_outputs, dtype=np.float32)
    mask = np.asarray(mask)
    f16 = np.float16
    Wqh, Wkh, Wvh = (np.asarray(w, np.float32) for w in (Wq, Wk, Wv))
    Wch = np.asarray(Wc, np.float32)

    idxs = [np.where(mask[b] == 0)[0] for b in range(B)]
    nbs = [len(i) for i in idxs]
    if max(nbs) <= 1024 + 64 and min(nbs) >= 896:
        cap, fused = CAP, 3    # attention q-range trimmed to 1088
    else:
        cap, fused = S, 0          # generic fallback: no compaction gain
    kt = cap // 128

    eyebar = (1.0 - np.eye(128, dtype=np.float32)).astype(f16)
    onesv = np.ones((1, 64), dtype=f16)

    in_maps = []
    for c in range(NCORES):
        b, half = c // 2, c % 2
        hsl = slice(half * PD, (half + 1) * PD)
        idx, nb = idxs[b], nbs[b]
        xc = np.zeros((cap, H), np.float32)
        xc[:nb] = x[b, idx[:nb]]
        xcT = np.ascontiguousarray(xc.T).astype(f16)
        padb = np.where(np.arange(cap) < nb, -1.0, -31.0).astype(np.float32)
        bqk = np.stack([np.asarray(bq, np.float32)[hsl].reshape(4, 128),
                        np.asarray(bk, np.float32)[hsl].reshape(4, 128)]
                       ).reshape(8, 128).T.copy()
        in_maps.append({
            "xcT": xcT,
            "wq": np.ascontiguousarray(Wqh[hsl, :].T).astype(f16),
            "wk": np.ascontiguousarray(Wkh[hsl, :].T).astype(f16),
            "wv": np.ascontiguousarray(Wvh[hsl, :].T).astype(f16),
            "wc": np.ascontiguousarray(Wch[:, hsl].T).astype(f16),
            "bqk": bqk,
            "padb": np.ascontiguousarray(padb.reshape(kt, 128).T),
            "eyebar": eyebar,
            "ones64": onesv,
        })

    res = run_bass_kernel_spmd(
        _get_nc(cap, fused), in_maps, list(range(NCORES)),
        trace=bool(os.environ.get("BASS_TRACE")))
    LAST_RESULTS = res

    bvWc = (np.asarray(bv, np.float64) @ np.asarray(Wc, np.float64).T
            ).astype(np.float32)
    out = np.zeros((B, S, H), dtype=np.float32)
    for b in range(B):
        nb = nbs[b]
        part = (res.results[2 * b]["out"][:nb].astype(np.float32)
                + res.results[2 * b + 1]["out"][:nb].astype(np.float32))
        out[b, idxs[b][:nb]] = part + bvWc[None, :]
    return out


# revision 27
# speedup vs baseline: 1.1807x; 1.0017x over previous
"""Multi-head self-attention TRN2 Bass kernel (v2).

Sharding: 8 cores = 4 batches x 2 head-halves. Core (b, half) computes
batch b with 8 heads (half*8 .. half*8+8), producing a [CAP, 1024]
partial of the output projection; the host sums the 2 halves per batch,
scatters compacted rows back to their original positions, and adds the
constant bv @ Wc.T row (softmax weights sum to 1, so the V-bias
contributes a constant vector).

Host-side compaction: only unmasked tokens (mask==0) take part in
attention -- masked queries produce zero rows and masked keys are
excluded. Tokens are compacted per batch and padded to CAP=1152
(valid counts are ~1004-1036), cutting attention work to (1152/2048)^2
and projection work to 1152/2048.

All matmuls run in fp16 (1 cycle/row on the PE at full clock, fast
weight loads); fp32r was 2-4x slower per row because it self-loads
weights and kept the PE at the cold 1.2 GHz p-state. Dots for the two
heads of a pair run concurrently via PE row-tiling (tile_position
(0,0)/(64,0), contraction 64 each).

Padding keys are killed with a per-partition bias on the exp
activation: p = exp(dots/8 - 1) for valid keys, exp(dots/8 - 31) ~ 0
for padded keys. The diagonal (self-attention) is zeroed by a DVE
multiply with (1-I) on the 128-column window that contains it. The
softmax denominator comes from a 65th all-ones column in the V'
stationary. Normalization: rep = broadcast(1/den) via a tiny
ones-stationary matmul, then one DVE multiply that also moves
O from PSUM to SBUF.
"""

import os
import numpy as np
import ml_dtypes

import concourse.bacc as bacc
import concourse.mybir as mybir
from concourse.tile import TileContext
from concourse.bass_utils import run_bass_kernel_spmd

B, S, H, NH, HD = 4, 2048, 1024, 16, 64
NCORES = 8
HPC = 8                   # heads per core
PD = HPC * HD             # per-core projection dim = 512
CAP = 1152                # compacted token capacity per batch
KT = CAP // 128           # 9 key tiles
FT = H // 128             # 8 feature k-tiles
QCH = [(0, 512), (512, 512), (1024, 128)]   # query chunks
F32 = mybir.dt.float32
F16 = mybir.dt.float16

LAST_RESULTS = None       # BassKernelResults from the most recent run


def build_bass(cap=CAP, fused_pairs=3):
    """fused_pairs: number of leading kt-pairs whose keys are known
    all-valid (bias -1 immediate, one exp instr per 2 key tiles).
    The remaining kt get per-kt exp instrs with a per-partition bias AP
    that encodes padding. fused_pairs=3 requires nb >= 896."""
    kt = cap // 128
    qch = [(o, n) for (o, n) in [(i * 512, min(512, cap - i * 512))
                                 for i in range((cap + 511) // 512)] if n > 0]
    if cap == CAP:
        # attention stages only need query columns < nb (max valid count
        # is ~1036); trim the last chunk 128 -> 64 (covers nb <= 1088,
        # checked at dispatch). Projections/outproj still cover all of cap.
        qch[-1] = (qch[-1][0], 64)
    nc = bacc.Bacc()
    xcT = nc.dram_tensor("xcT", [H, cap], F16, kind="ExternalInput")
    wq = nc.dram_tensor("wq", [H, PD], F16, kind="ExternalInput")
    wk = nc.dram_tensor("wk", [H, PD], F16, kind="ExternalInput")
    wv = nc.dram_tensor("wv", [H, PD], F16, kind="ExternalInput")
    wc = nc.dram_tensor("wc", [PD, H], F16, kind="ExternalInput")
    bqk = nc.dram_tensor("bqk", [128, 8], F32, kind="ExternalInput")
    padb = nc.dram_tensor("padb", [128, kt], F32, kind="ExternalInput")
    eyebar = nc.dram_tensor("eyebar", [128, 128], F16, kind="ExternalInput")
    ones64 = nc.dram_tensor("ones64", [128, 64], F16, kind="ExternalInput")
    outp = nc.dram_tensor("out", [cap, H], F16, kind="ExternalOutput")

    EXP = mybir.ActivationFunctionType.Exp
    DR = None  # no DoubleRow (fp16 path)

    # exp schedule: list of (kts, bias_mode); bias_mode None -> -1.0 imm
    sched = []
    for i in range(fused_pairs):
        sched.append(((2 * i, 2 * i + 1), None))
    for k in range(2 * fused_pairs, kt):
        sched.append(((k,), k))
    # which kt are known all-valid without AP bias
    for (kts, bm) in sched:
        if bm is None:
            assert max(kts) < 2 * fused_pairs

    with TileContext(nc) as tc, \
         tc.tile_pool(name="consts", bufs=1) as cpool, \
         tc.tile_pool(name="work", bufs=1) as wpool, \
         tc.tile_pool(name="ptp", bufs=4) as ppool, \
         tc.tile_pool(name="small", bufs=3) as spool, \
         tc.tile_pool(name="osb", bufs=2) as opool, \
         tc.tile_pool(name="psum", bufs=1, space="PSUM") as pspool:

        # ---- constants / weights ----
        w_sb = {}
        for name, t in (("q", wq), ("k", wk), ("v", wv)):
            sb = cpool.tile([128, FT, PD], F16, name=f"w{name}sb")
            for ft in range(FT):
                nc.sync.dma_start(out=sb[:, ft, :],
                                  in_=t[ft * 128:(ft + 1) * 128, :])
            w_sb[name] = sb
        wc_sb = cpool.tile([128, 4, H], F16, name="wcsb")
        for g in range(4):
            nc.sync.dma_start(out=wc_sb[:, g, :],
                              in_=wc[g * 128:(g + 1) * 128, :])
        bqk_sb = cpool.tile([128, 8], F32, name="bqksb")
        nc.sync.dma_start(out=bqk_sb[:, :], in_=bqk[:, :])
        padb_sb = cpool.tile([128, kt], F32, name="padbsb")
        nc.sync.dma_start(out=padb_sb[:, :], in_=padb[:, :])
        eye_sb = cpool.tile([128, 128], F16, name="eyesb")
        nc.sync.dma_start(out=eye_sb[:, :], in_=eyebar[:, :])
        ones_sb = cpool.tile([128, 64], F16, name="onessb")
        nc.sync.dma_start(out=ones_sb[:, :], in_=ones64[:, :])

        xc_sb = wpool.tile([128, FT, cap], F16, name="xcsb")
        for ft in range(FT):
            nc.sync.dma_start(out=xc_sb[:, ft, :],
                              in_=xcT[ft * 128:(ft + 1) * 128, :])

        # ---- q/k projections -> qkT [128, 4, cap] f16 ----
        qkT = {w: wpool.tile([128, 4, cap], F16, name=f"{w}T")
               for w in "qk"}
        for wi, w in enumerate("qk"):
            for g in range(4):
                for qo, qn in qch:
                    pp = pspool.tile([128, 512], F32, tag="mm", bufs=2)
                    for ft in range(FT):
                        nc.tensor.matmul(
                            pp[:, 0:qn],
                            w_sb[w][:, ft, g * 128:(g + 1) * 128],
                            xc_sb[:, ft, qo:qo + qn],
                            start=(ft == 0), stop=(ft == FT - 1))
                    nc.vector.tensor_scalar_add(
                        qkT[w][:, g, qo:qo + qn], pp[:, 0:qn],
                        bqk_sb[:, 4 * wi + g:4 * wi + g + 1])

        # ---- v projection (token-major) -> vp [128, kt, 8, 65] f16 ----
        # vp[key, kt, h, m] = V[key, h*64+m]; vp[key, kt, h, 64] = 1 (denom)
        vp = wpool.tile([128, kt, HPC, 65], F16, name="vp")
        for tt in range(kt):
            pv = pspool.tile([128, 512], F32, tag="mm", bufs=2)
            for ft in range(FT):
                nc.tensor.matmul(
                    pv[:, :],
                    xc_sb[:, ft, tt * 128:(tt + 1) * 128],
                    w_sb["v"][:, ft, :],
                    start=(ft == 0), stop=(ft == FT - 1))
            # strided copy: [128, 8, 64] from pv into the 65-strided vp
            nc.vector.tensor_copy(
                vp[:, tt, :, 0:64],
                pv.rearrange("p (h d) -> p h d", d=64))
        nc.vector.memset(vp[:, :, :, 64:65], 1.0)

        # ---- attention, 3-stage software pipeline over (pair, qchunk) ----
        # stage 1: dots + exp + diag -> ptp tiles
        # stage 2 (one step behind): AV accumulation + reciprocal issue
        # stage 3 (two steps behind): rep broadcast + normalize
        # This keeps the PE fed: exp(i) overlaps AV(i-1) on the PE, and the
        # slow [1,N] reciprocal gets a full step of slack before rep reads it.
        onorm = wpool.tile([128, 4, cap], F16, name="onorm")
        avq = []        # (ptp pair, p, qo, qn) awaiting AV
        deferred = []   # (osb, rcref, row, p, rb, qo, qn) awaiting normalize
        den_st = {"tile": None, "row": 0, "rc": None}

        def flush_norm(count):
            for _ in range(count):
                if not deferred or deferred[0][1]["rc"] is None:
                    return
                osb_d, rcref, row, p_d, rb_d, qo_d, qn_d = deferred.pop(0)
                rc16 = rcref["rc"]
                rep = pspool.tile([64, 512], F32, tag="mm", bufs=2,
                                  name="rep")
                nc.tensor.matmul(rep[:, 0:qn_d],
                                 ones_sb[32 * row:32 * row + 1, :],
                                 rc16[32 * row:32 * row + 1, 0:qn_d],
                                 start=True, stop=True,
                                 tile_position=(32 * row, 0))
                nc.vector.tensor_mul(
                    onorm[rb_d:rb_d + 64, p_d, qo_d:qo_d + qn_d],
                    osb_d[:, 0:qn_d], rep[:, 0:qn_d])

        def do_av(ptp_a, p_a, qo_a, qn_a):
            # normalize two steps-old entries; their rc is long done
            flush_norm(2)
            for h01 in range(2):
                h = 2 * p_a + h01
                rb = h01 * 64
                avs = pspool.tile([65, 512], F32, tag="avs", bufs=2,
                                  name="avs")
                for k in range(kt):
                    nc.tensor.matmul(
                        avs[:, 0:qn_a],
                        vp[:, k, h, :],
                        ptp_a[h01][:, k, 0:qn_a],
                        start=(k == 0), stop=(k == kt - 1),
                        perf_mode=DR)
                # free avs early: O (f16) and den row out to SBUF
                osb = spool.tile([64, 512], F16, tag="osb", bufs=6,
                                 name="oun")
                nc.vector.tensor_copy(osb[:, 0:qn_a], avs[0:64, 0:qn_a])
                if den_st["tile"] is None:
                    den_st["tile"] = spool.tile([128, 512], F32,
                                                tag="den4", bufs=2,
                                                name="den4")
                    nc.vector.memset(den_st["tile"][:, :], 1.0)
                    den_st["row"] = 0
                    den_st["rc"] = {"rc": None}
                d4, row = den_st["tile"], den_st["row"]
                nc.vector.tensor_copy(d4[32 * row:32 * row + 1, 0:qn_a],
                                      avs[64:65, 0:qn_a])
                deferred.append((osb, den_st["rc"], row, p_a, rb,
                                 qo_a, qn_a))
                den_st["row"] += 1
                if den_st["row"] == 4:
                    # one reciprocal covers 4 denominators (4 DVE lanes
                    # run in parallel; [1,N] and [4,N] cost the same)
                    rc32 = spool.tile([128, 512], F32, tag="rc32",
                                      bufs=2, name="rc32")
                    with nc.allow_low_precision(
                            reason="1/den in fp16: 0.05% rel, den>=13"):
                        nc.vector.reciprocal(rc32[0:97, :], d4[0:97, :])
                    rc16 = spool.tile([128, 512], F16, tag="rc16", bufs=2,
                                      name="rc16")
                    nc.vector.tensor_copy(rc16[0:97, :], rc32[0:97, :])
                    den_st["rc"]["rc"] = rc16
                    den_st["tile"] = None

        for p in range(4):
            for qo, qn in qch:
                # AV of the previous step first: its reciprocal lands in
                # the DVE queue ahead of this step's diag multiplies
                # (which wait on the exp tail)
                if avq:
                    do_av(*avq.pop(0))
                ptp = [ppool.tile([128, kt, 512], F16, tag=f"ptp{h01}",
                                  bufs=2, name=f"ptp{p}{h01}{qo}")
                       for h01 in range(2)]
                for h01 in range(2):
                    rb = h01 * 64
                    for kts, bm in sched:
                        dp = pspool.tile([128, 2, 512], F32, tag="dp", bufs=2)
                        for j, k in enumerate(kts):
                            nc.tensor.matmul(
                                dp[:, j, 0:qn],
                                qkT["k"][rb:rb + 64, p,
                                         k * 128:(k + 1) * 128],
                                qkT["q"][rb:rb + 64, p, qo:qo + qn],
                                start=True, stop=True,
                                tile_position=(rb, 0))
                        # fused groups: all keys valid -> their padb col
                        # is uniformly -1, so kts[0]'s column works
                        bias = padb_sb[:, kts[0]:kts[0] + 1] if bm is None \
                            else padb_sb[:, bm:bm + 1]
                        nc.scalar.activation(
                            ptp[h01][:, kts[0]:kts[0] + len(kts), 0:qn],
                            dp[:, 0:len(kts), 0:qn],
                            EXP, scale=0.125, bias=bias)
                    # diagonal zeroing for kt windows inside this q chunk
                    k0, k1 = qo // 128, -(-(qo + qn) // 128)
                    for k in range(k0, k1):
                        off = k * 128 - qo
                        w = min(qn - off, 128)
                        nc.vector.tensor_mul(
                            ptp[h01][:, k, off:off + w],
                            ptp[h01][:, k, off:off + w],
                            eye_sb[:, 0:w])
                avq.append((ptp, p, qo, qn))
        while avq:
            do_av(*avq.pop(0))
        flush_norm(len(deferred))

        # ---- output projection ----
        for tt in range(kt):
            osb = opool.tile([128, H], F16, tag="osb")
            for oc in range(2):
                op = pspool.tile([128, 512], F32, tag="mm", bufs=2)
                for g in range(4):
                    nc.tensor.matmul(
                        op[:, :],
                        onorm[:, g, tt * 128:(tt + 1) * 128],
                        wc_sb[:, g, oc * 512:(oc + 1) * 512],
                        start=(g == 0), stop=(g == 3))
                nc.vector.tensor_copy(osb[:, oc * 512:(oc + 1) * 512],
                                      op[:, :])
            nc.sync.dma_start(
                out=outp[tt * 128:(tt + 1) * 128, :], in_=osb[:, :])
    nc.finalize()
    return nc


_NC_CACHE = {}


def _get_nc(cap, fused_pairs):
    key = (cap, fused_pairs)
    if key not in _NC_CACHE:
        _NC_CACHE[key] = build_bass(cap, fused_pairs)
    return _NC_CACHE[key]


def kernel(encoder_outputs, mask, Wq, bq, Wk, bk, Wv, bv, Wc):
    global LAST_RESULTS
    x = np.asarray(encoder_outputs, dtype=np.float32)
    mask = np.asarray(mask)
    f16 = np.float16
    Wqh, Wkh, Wvh = (np.asarray(w, np.float32) for w in (Wq, Wk, Wv))
    Wch = np.asarray(Wc, np.float32)

    idxs = [np.where(mask[b] == 0)[0] for b in range(B)]
    nbs = [len(i) for i in idxs]
    if max(nbs) <= 1024 + 64 and min(nbs) >= 896:
        cap, fused = CAP, 3    # attention q-range trimmed to 1088
    else:
        cap, fused = S, 0          # generic fallback: no compaction gain
    kt = cap // 128

    eyebar = (1.0 - np.eye(128, dtype=np.float32)).astype(f16)
    onesv = np.ones((128, 64), dtype=f16)

    in_maps = []
    for c in range(NCORES):
        b, half = c // 2, c % 2
        hsl = slice(half * PD, (half + 1) * PD)
        idx, nb = idxs[b], nbs[b]
        xc = np.zeros((cap, H), np.float32)
        xc[:nb] = x[b, idx[:nb]]
        xcT = np.ascontiguousarray(xc.T).astype(f16)
        padb = np.where(np.arange(cap) < nb, -1.0, -31.0).astype(np.float32)
        bqk = np.stack([np.asarray(bq, np.float32)[hsl].reshape(4, 128),
                        np.asarray(bk, np.float32)[hsl].reshape(4, 128)]
                       ).reshape(8, 128).T.copy()
        in_maps.append({
            "xcT": xcT,
            "wq": np.ascontiguousarray(Wqh[hsl, :].T).astype(f16),
            "wk": np.ascontiguousarray(Wkh[hsl, :].T).astype(f16),
            "wv": np.ascontiguousarray(Wvh[hsl, :].T).astype(f16),
            "wc": np.ascontiguousarray(Wch[:, hsl].T).astype(f16),
            "bqk": bqk,
            "padb": np.ascontiguousarray(padb.reshape(kt, 128).T),
            "eyebar": eyebar,
            "ones64": onesv,
        })

    res = run_bass_kernel_spmd(
        _get_nc(cap, fused), in_maps, list(range(NCORES)),
        trace=bool(os.environ.get("BASS_TRACE")))
    LAST_RESULTS = res

    bvWc = (np.asarray(bv, np.float64) @ np.asarray(Wc, np.float64).T
            ).astype(np.float32)
    out = np.zeros((B, S, H), dtype=np.float32)
    for b in range(B):
        nb = nbs[b]
        part = (res.results[2 * b]["out"][:nb].astype(np.float32)
                + res.results[2 * b + 1]["out"][:nb].astype(np.float32))
        out[b, idxs[b][:nb]] = part + bvWc[None, :]
    return out


# revision 30
# speedup vs baseline: 1.3724x; 1.1624x over previous
"""Multi-head self-attention TRN2 Bass kernel (v2).

Sharding: 8 cores = 4 batches x 2 head-halves. Core (b, half) computes
batch b with 8 heads (half*8 .. half*8+8), producing a [CAP, 1024]
partial of the output projection; the host sums the 2 halves per batch,
scatters compacted rows back to their original positions, and adds the
constant bv @ Wc.T row (softmax weights sum to 1, so the V-bias
contributes a constant vector).

Host-side compaction: only unmasked tokens (mask==0) take part in
attention -- masked queries produce zero rows and masked keys are
excluded. Tokens are compacted per batch and padded to CAP=1152
(valid counts are ~1004-1036), cutting attention work to (1152/2048)^2
and projection work to 1152/2048.

All matmuls run in fp16 (1 cycle/row on the PE at full clock, fast
weight loads); fp32r was 2-4x slower per row because it self-loads
weights and kept the PE at the cold 1.2 GHz p-state. Dots for the two
heads of a pair run concurrently via PE row-tiling (tile_position
(0,0)/(64,0), contraction 64 each).

Padding keys are killed with a per-partition bias on the exp
activation: p = exp(dots/8 - 1) for valid keys, exp(dots/8 - 31) ~ 0
for padded keys. The diagonal (self-attention) is zeroed by a DVE
multiply with (1-I) on the 128-column window that contains it. The
softmax denominator comes from a 65th all-ones column in the V'
stationary. Normalization: rep = broadcast(1/den) via a tiny
ones-stationary matmul, then one DVE multiply that also moves
O from PSUM to SBUF.
"""

import os
import numpy as np
import ml_dtypes

import concourse.bacc as bacc
import concourse.mybir as mybir
from concourse.tile import TileContext
from concourse.bass_utils import run_bass_kernel_spmd

B, S, H, NH, HD = 4, 2048, 1024, 16, 64
NCORES = 8
HPC = 8                   # heads per core
PD = HPC * HD             # per-core projection dim = 512
CAP = 1152                # compacted token capacity per batch
KT = CAP // 128           # 9 key tiles
FT = H // 128             # 8 feature k-tiles
QCH = [(0, 512), (512, 512), (1024, 128)]   # query chunks
F32 = mybir.dt.float32
F16 = mybir.dt.float16

LAST_RESULTS = None       # BassKernelResults from the most recent run


def build_bass(cap=CAP, fused_pairs=3):
    """fused_pairs: number of leading kt-pairs whose keys are known
    all-valid (bias -1 immediate, one exp instr per 2 key tiles).
    The remaining kt get per-kt exp instrs with a per-partition bias AP
    that encodes padding. fused_pairs=3 requires nb >= 896."""
    kt = cap // 128
    qch = [(o, n) for (o, n) in [(i * 512, min(512, cap - i * 512))
                                 for i in range((cap + 511) // 512)] if n > 0]
    if cap == CAP:
        # attention stages only need query columns < nb (max valid count
        # is ~1036); trim the last chunk 128 -> 64 (covers nb <= 1088,
        # checked at dispatch). Projections/outproj still cover all of cap.
        qch[-1] = (qch[-1][0], 64)
    nc = bacc.Bacc()
    xcT = nc.dram_tensor("xcT", [H, cap], F16, kind="ExternalInput")
    wq = nc.dram_tensor("wq", [H, PD], F16, kind="ExternalInput")
    wk = nc.dram_tensor("wk", [H, PD], F16, kind="ExternalInput")
    wv = nc.dram_tensor("wv", [H, PD], F16, kind="ExternalInput")
    wc = nc.dram_tensor("wc", [PD, H], F16, kind="ExternalInput")
    bqk = nc.dram_tensor("bqk", [128, 8], F32, kind="ExternalInput")
    padb = nc.dram_tensor("padb", [128, kt], F32, kind="ExternalInput")
    eyebar = nc.dram_tensor("eyebar", [128, 128], F16, kind="ExternalInput")
    inv16 = nc.dram_tensor("inv16", [128, kt], F32, kind="ExternalInput")
    ones64 = nc.dram_tensor("ones64", [128, 64], F16, kind="ExternalInput")
    outp = nc.dram_tensor("out", [cap, H], F16, kind="ExternalOutput")

    EXP = mybir.ActivationFunctionType.Exp
    DR = None  # no DoubleRow (fp16 path)

    # exp schedule: padding keys are masked via zeroed V'/ones columns
    # (not exp bias), so every exp instr uses the constant -1 bias and
    # key tiles fuse uniformly into pairs
    sched = [((2 * i, 2 * i + 1), None) for i in range(kt // 2)]
    if kt % 2:
        sched.append(((kt - 1,), None))

    with TileContext(nc) as tc, \
         tc.tile_pool(name="consts", bufs=1) as cpool, \
         tc.tile_pool(name="work", bufs=1) as wpool, \
         tc.tile_pool(name="ptp", bufs=4) as ppool, \
         tc.tile_pool(name="small", bufs=3) as spool, \
         tc.tile_pool(name="osb", bufs=2) as opool, \
         tc.tile_pool(name="psum", bufs=1, space="PSUM") as pspool:

        # ---- constants / weights ----
        w_sb = {}
        for name, t in (("q", wq), ("k", wk), ("v", wv)):
            sb = cpool.tile([128, FT, PD], F16, name=f"w{name}sb")
            for ft in range(FT):
                nc.sync.dma_start(out=sb[:, ft, :],
                                  in_=t[ft * 128:(ft + 1) * 128, :])
            w_sb[name] = sb
        wc_sb = cpool.tile([128, 4, H], F16, name="wcsb")
        for g in range(4):
            nc.sync.dma_start(out=wc_sb[:, g, :],
                              in_=wc[g * 128:(g + 1) * 128, :])
        bqk_sb = cpool.tile([128, 8], F32, name="bqksb")
        nc.sync.dma_start(out=bqk_sb[:, :], in_=bqk[:, :])
        padb_sb = cpool.tile([128, kt], F32, name="padbsb")
        nc.sync.dma_start(out=padb_sb[:, :], in_=padb[:, :])
        eye_sb = cpool.tile([128, 128], F16, name="eyesb")
        nc.sync.dma_start(out=eye_sb[:, :], in_=eyebar[:, :])
        inv_sb = cpool.tile([128, kt], F32, name="invsb")
        nc.sync.dma_start(out=inv_sb[:, :], in_=inv16[:, :])
        ones_sb = cpool.tile([128, 64], F16, name="onessb")
        nc.sync.dma_start(out=ones_sb[:, :], in_=ones64[:, :])

        xc_sb = wpool.tile([128, FT, cap], F16, name="xcsb")
        for ft in range(FT):
            nc.sync.dma_start(out=xc_sb[:, ft, :],
                              in_=xcT[ft * 128:(ft + 1) * 128, :])

        # ---- q/k projections -> qkT [128, 4, cap] f16 ----
        qkT = {w: wpool.tile([128, 4, cap], F16, name=f"{w}T")
               for w in "qk"}
        for wi, w in enumerate("qk"):
            for g in range(4):
                for qo, qn in qch:
                    pp = pspool.tile([128, 512], F32, tag="mm", bufs=2)
                    for ft in range(FT):
                        nc.tensor.matmul(
                            pp[:, 0:qn],
                            w_sb[w][:, ft, g * 128:(g + 1) * 128],
                            xc_sb[:, ft, qo:qo + qn],
                            start=(ft == 0), stop=(ft == FT - 1))
                    nc.vector.tensor_scalar_add(
                        qkT[w][:, g, qo:qo + qn], pp[:, 0:qn],
                        bqk_sb[:, 4 * wi + g:4 * wi + g + 1])

        # ---- v projection (token-major) -> vp [128, kt, 8, 65] f16 ----
        # vp[key, kt, h, m] = V[key, h*64+m]; vp[key, kt, h, 64] = 1 (denom)
        vp = wpool.tile([128, kt, HPC, 65], F16, name="vp")
        for tt in range(kt):
            pv = pspool.tile([128, 512], F32, tag="mm", bufs=2)
            for ft in range(FT):
                nc.tensor.matmul(
                    pv[:, :],
                    xc_sb[:, ft, tt * 128:(tt + 1) * 128],
                    w_sb["v"][:, ft, :],
                    start=(ft == 0), stop=(ft == FT - 1))
            # strided copy with padding-key mask: [128, 8, 64] from pv
            nc.vector.tensor_scalar_mul(
                vp[:, tt, :, 0:64],
                pv.rearrange("p (h d) -> p h d", d=64),
                inv_sb[:, tt:tt + 1])
            nc.vector.tensor_copy(
                vp[:, tt, :, 64:65],
                inv_sb[:, tt:tt + 1].to_broadcast((128, HPC, 1)))

        # ---- attention, 3-stage software pipeline over (pair, qchunk) ----
        # stage 1: dots + exp + diag -> ptp tiles
        # stage 2 (one step behind): AV accumulation + reciprocal issue
        # stage 3 (two steps behind): rep broadcast + normalize
        # This keeps the PE fed: exp(i) overlaps AV(i-1) on the PE, and the
        # slow [1,N] reciprocal gets a full step of slack before rep reads it.
        onorm = wpool.tile([128, 4, cap], F16, name="onorm")
        avq = []        # (ptp pair, p, qo, qn) awaiting AV
        deferred = []   # (osb, rcref, row, p, rb, qo, qn) awaiting normalize
        den_st = {"tile": None, "row": 0, "rc": None}

        def flush_norm(count):
            for _ in range(count):
                if not deferred or deferred[0][1]["rc"] is None:
                    return
                osb_d, rcref, row, p_d, rb_d, qo_d, qn_d = deferred.pop(0)
                rc16 = rcref["rc"]
                rep = pspool.tile([64, 512], F32, tag="mm", bufs=2,
                                  name="rep")
                nc.tensor.matmul(rep[:, 0:qn_d],
                                 ones_sb[32 * row:32 * row + 1, :],
                                 rc16[32 * row:32 * row + 1, 0:qn_d],
                                 start=True, stop=True,
                                 tile_position=(32 * row, 0))
                nc.vector.tensor_mul(
                    onorm[rb_d:rb_d + 64, p_d, qo_d:qo_d + qn_d],
                    osb_d[:, 0:qn_d], rep[:, 0:qn_d])

        def do_av(ptp_a, p_a, qo_a, qn_a):
            # normalize two steps-old entries; their rc is long done
            flush_norm(2)
            for h01 in range(2):
                h = 2 * p_a + h01
                rb = h01 * 64
                avs = pspool.tile([65, 512], F32, tag="avs", bufs=2,
                                  name="avs")
                for k in range(kt):
                    nc.tensor.matmul(
                        avs[:, 0:qn_a],
                        vp[:, k, h, :],
                        ptp_a[:, k, h01, 0:qn_a],
                        start=(k == 0), stop=(k == kt - 1),
                        perf_mode=DR)
                # free avs early: O (f16) and den row out to SBUF
                osb = spool.tile([64, 512], F16, tag="osb", bufs=6,
                                 name="oun")
                nc.vector.tensor_copy(osb[:, 0:qn_a], avs[0:64, 0:qn_a])
                if den_st["tile"] is None:
                    den_st["tile"] = spool.tile([128, 512], F32,
                                                tag="den4", bufs=2,
                                                name="den4")
                    nc.vector.memset(den_st["tile"][:, :], 1.0)
                    den_st["row"] = 0
                    den_st["rc"] = {"rc": None}
                d4, row = den_st["tile"], den_st["row"]
                nc.vector.tensor_copy(d4[32 * row:32 * row + 1, 0:qn_a],
                                      avs[64:65, 0:qn_a])
                deferred.append((osb, den_st["rc"], row, p_a, rb,
                                 qo_a, qn_a))
                den_st["row"] += 1
                if den_st["row"] == 4:
                    # one reciprocal covers 4 denominators (4 DVE lanes
                    # run in parallel; [1,N] and [4,N] cost the same)
                    rc32 = spool.tile([128, 512], F32, tag="rc32",
                                      bufs=2, name="rc32")
                    with nc.allow_low_precision(
                            reason="1/den in fp16: 0.05% rel, den>=13"):
                        nc.vector.reciprocal(rc32[0:97, :], d4[0:97, :])
                    rc16 = spool.tile([128, 512], F16, tag="rc16", bufs=2,
                                      name="rc16")
                    nc.vector.tensor_copy(rc16[0:97, :], rc32[0:97, :])
                    den_st["rc"]["rc"] = rc16
                    den_st["tile"] = None

        for p in range(4):
            for qo, qn in qch:
                # AV of the previous step first: its reciprocal lands in
                # the DVE queue ahead of this step's diag multiplies
                # (which wait on the exp tail)
                if avq:
                    do_av(*avq.pop(0))
                ptp = ppool.tile([128, kt, 2, 512], F16, tag="ptp",
                                 bufs=2, name=f"ptp{p}{qo}")
                for k in range(kt):
                    # both heads' dots adjacent at row groups 0/64 -> the
                    # PE runs them concurrently; one exp covers both
                    dp = pspool.tile([128, 2, 512], F32, tag="dp", bufs=2)
                    for h01 in range(2):
                        rb = h01 * 64
                        nc.tensor.matmul(
                            dp[:, h01, 0:qn],
                            qkT["k"][rb:rb + 64, p,
                                     k * 128:(k + 1) * 128],
                            qkT["q"][rb:rb + 64, p, qo:qo + qn],
                            start=True, stop=True,
                            tile_position=(rb, 0))
                    nc.scalar.activation(
                        ptp[:, k, :, 0:qn], dp[:, :, 0:qn],
                        EXP, scale=0.125, bias=padb_sb[:, 0:1])
                # diagonal zeroing for kt windows inside this q chunk
                k0, k1 = qo // 128, -(-(qo + qn) // 128)
                for k in range(k0, k1):
                    off = k * 128 - qo
                    w = min(qn - off, 128)
                    for h01 in range(2):
                        nc.vector.tensor_mul(
                            ptp[:, k, h01, off:off + w],
                            ptp[:, k, h01, off:off + w],
                            eye_sb[:, 0:w])
                avq.append((ptp, p, qo, qn))
        while avq:
            do_av(*avq.pop(0))
        flush_norm(len(deferred))

        # ---- output projection ----
        for tt in range(kt):
            osb = opool.tile([128, H], F16, tag="osb")
            for oc in range(2):
                op = pspool.tile([128, 512], F32, tag="mm", bufs=2)
                for g in range(4):
                    nc.tensor.matmul(
                        op[:, :],
                        onorm[:, g, tt * 128:(tt + 1) * 128],
                        wc_sb[:, g, oc * 512:(oc + 1) * 512],
                        start=(g == 0), stop=(g == 3))
                nc.vector.tensor_copy(osb[:, oc * 512:(oc + 1) * 512],
                                      op[:, :])
            nc.sync.dma_start(
                out=outp[tt * 128:(tt + 1) * 128, :], in_=osb[:, :])
    nc.finalize()
    return nc


_NC_CACHE = {}


def _get_nc(cap, fused_pairs):
    key = (cap, fused_pairs)
    if key not in _NC_CACHE:
        _NC_CACHE[key] = build_bass(cap, fused_pairs)
    return _NC_CACHE[key]


def kernel(encoder_outputs, mask, Wq, bq, Wk, bk, Wv, bv, Wc):
    global LAST_RESULTS
    x = np.asarray(encoder_outputs, dtype=np.float32)
    mask = np.asarray(mask)
    f16 = np.float16
    Wqh, Wkh, Wvh = (np.asarray(w, np.float32) for w in (Wq, Wk, Wv))
    Wch = np.asarray(Wc, np.float32)

    idxs = [np.where(mask[b] == 0)[0] for b in range(B)]
    nbs = [len(i) for i in idxs]
    if max(nbs) <= 1024 + 64 and min(nbs) >= 896:
        cap, fused = CAP, 3    # attention q-range trimmed to 1088
    else:
        cap, fused = S, 0          # generic fallback: no compaction gain
    kt = cap // 128

    eyebar = (1.0 - np.eye(128, dtype=np.float32)).astype(f16)
    onesv = np.ones((128, 64), dtype=f16)

    in_maps = []
    for c in range(NCORES):
        b, half = c // 2, c % 2
        hsl = slice(half * PD, (half + 1) * PD)
        idx, nb = idxs[b], nbs[b]
        xc = np.zeros((cap, H), np.float32)
        xc[:nb] = x[b, idx[:nb]]
        xcT = np.ascontiguousarray(xc.T).astype(f16)
        padb = np.full(cap, -1.0, dtype=np.float32)
        inv = (np.arange(cap) < nb).astype(np.float32)
        bqk = np.stack([np.asarray(bq, np.float32)[hsl].reshape(4, 128),
                        np.asarray(bk, np.float32)[hsl].reshape(4, 128)]
                       ).reshape(8, 128).T.copy()
        in_maps.append({
            "xcT": xcT,
            "wq": np.ascontiguousarray(Wqh[hsl, :].T).astype(f16),
            "wk": np.ascontiguousarray(Wkh[hsl, :].T).astype(f16),
            "wv": np.ascontiguousarray(Wvh[hsl, :].T).astype(f16),
            "wc": np.ascontiguousarray(Wch[:, hsl].T).astype(f16),
            "bqk": bqk,
            "padb": np.ascontiguousarray(padb.reshape(kt, 128).T),
            "eyebar": eyebar,
            "inv16": np.ascontiguousarray(
                inv.reshape(kt, 128).T.astype(np.float32)),
            "ones64": onesv,
        })

    res = run_bass_kernel_spmd(
        _get_nc(cap, fused), in_maps, list(range(NCORES)),
        trace=bool(os.environ.get("BASS_TRACE")))
    LAST_RESULTS = res

    bvWc = (np.asarray(bv, np.float64) @ np.asarray(Wc, np.float64).T
            ).astype(np.float32)
    out = np.zeros((B, S, H), dtype=np.float32)
    for b in range(B):
        nb = nbs[b]
        part = (res.results[2 * b]["out"][:nb].astype(np.float32)
                + res.results[2 * b + 1]["out"][:nb].astype(np.float32))
        out[b, idxs[b][:nb]] = part + bvWc[None, :]
    return out


# revision 31
# speedup vs baseline: 1.4016x; 1.0213x over previous
"""Multi-head self-attention TRN2 Bass kernel (v2).

Sharding: 8 cores = 4 batches x 2 head-halves. Core (b, half) computes
batch b with 8 heads (half*8 .. half*8+8), producing a [CAP, 1024]
partial of the output projection; the host sums the 2 halves per batch,
scatters compacted rows back to their original positions, and adds the
constant bv @ Wc.T row (softmax weights sum to 1, so the V-bias
contributes a constant vector).

Host-side compaction: only unmasked tokens (mask==0) take part in
attention -- masked queries produce zero rows and masked keys are
excluded. Tokens are compacted per batch and padded to CAP=1152
(valid counts are ~1004-1036), cutting attention work to (1152/2048)^2
and projection work to 1152/2048.

All matmuls run in fp16 (1 cycle/row on the PE at full clock, fast
weight loads); fp32r was 2-4x slower per row because it self-loads
weights and kept the PE at the cold 1.2 GHz p-state. Dots for the two
heads of a pair run concurrently via PE row-tiling (tile_position
(0,0)/(64,0), contraction 64 each).

Padding keys are killed with a per-partition bias on the exp
activation: p = exp(dots/8 - 1) for valid keys, exp(dots/8 - 31) ~ 0
for padded keys. The diagonal (self-attention) is zeroed by a DVE
multiply with (1-I) on the 128-column window that contains it. The
softmax denominator comes from a 65th all-ones column in the V'
stationary. Normalization: rep = broadcast(1/den) via a tiny
ones-stationary matmul, then one DVE multiply that also moves
O from PSUM to SBUF.
"""

import os
import numpy as np
import ml_dtypes

import concourse.bacc as bacc
import concourse.mybir as mybir
from concourse.tile import TileContext
from concourse.bass_utils import run_bass_kernel_spmd

B, S, H, NH, HD = 4, 2048, 1024, 16, 64
NCORES = 8
HPC = 8                   # heads per core
PD = HPC * HD             # per-core projection dim = 512
CAP = 1152                # compacted token capacity per batch
KT = CAP // 128           # 9 key tiles
FT = H // 128             # 8 feature k-tiles
QCH = [(0, 512), (512, 512), (1024, 128)]   # query chunks
F32 = mybir.dt.float32
F16 = mybir.dt.float16

LAST_RESULTS = None       # BassKernelResults from the most recent run


def build_bass(cap=CAP, fused_pairs=3):
    """fused_pairs: number of leading kt-pairs whose keys are known
    all-valid (bias -1 immediate, one exp instr per 2 key tiles).
    The remaining kt get per-kt exp instrs with a per-partition bias AP
    that encodes padding. fused_pairs=3 requires nb >= 896."""
    kt = cap // 128
    qch = [(o, n) for (o, n) in [(i * 512, min(512, cap - i * 512))
                                 for i in range((cap + 511) // 512)] if n > 0]
    if cap == CAP:
        # attention stages only need query columns < nb (max valid count
        # is ~1036); trim the last chunk 128 -> 64 (covers nb <= 1088,
        # checked at dispatch). Projections/outproj still cover all of cap.
        qch[-1] = (qch[-1][0], 64)
    nc = bacc.Bacc()
    xcT = nc.dram_tensor("xcT", [H, cap], F16, kind="ExternalInput")
    wq = nc.dram_tensor("wq", [H, PD], F16, kind="ExternalInput")
    wk = nc.dram_tensor("wk", [H, PD], F16, kind="ExternalInput")
    wv = nc.dram_tensor("wv", [H, PD], F16, kind="ExternalInput")
    wc = nc.dram_tensor("wc", [PD, H], F16, kind="ExternalInput")
    bqk = nc.dram_tensor("bqk", [128, 8], F32, kind="ExternalInput")
    padb = nc.dram_tensor("padb", [128, kt], F32, kind="ExternalInput")
    eyebar = nc.dram_tensor("eyebar", [128, 128], F16, kind="ExternalInput")
    inv16 = nc.dram_tensor("inv16", [128, kt], F32, kind="ExternalInput")
    ones64 = nc.dram_tensor("ones64", [128, 64], F16, kind="ExternalInput")
    outp = nc.dram_tensor("out", [cap, H], F16, kind="ExternalOutput")

    EXP = mybir.ActivationFunctionType.Exp
    DR = None  # no DoubleRow (fp16 path)

    # exp schedule: padding keys are masked via zeroed V'/ones columns
    # (not exp bias), so every exp instr uses the constant -1 bias and
    # key tiles fuse uniformly into pairs
    sched = [((2 * i, 2 * i + 1), None) for i in range(kt // 2)]
    if kt % 2:
        sched.append(((kt - 1,), None))

    with TileContext(nc) as tc, \
         tc.tile_pool(name="consts", bufs=1) as cpool, \
         tc.tile_pool(name="work", bufs=1) as wpool, \
         tc.tile_pool(name="ptp", bufs=4) as ppool, \
         tc.tile_pool(name="small", bufs=3) as spool, \
         tc.tile_pool(name="osb", bufs=2) as opool, \
         tc.tile_pool(name="psum", bufs=1, space="PSUM") as pspool:

        # ---- constants / weights ----
        w_sb = {}
        for name, t in (("q", wq), ("k", wk), ("v", wv)):
            sb = cpool.tile([128, FT, PD], F16, name=f"w{name}sb")
            for ft in range(FT):
                nc.sync.dma_start(out=sb[:, ft, :],
                                  in_=t[ft * 128:(ft + 1) * 128, :])
            w_sb[name] = sb
        wc_sb = cpool.tile([128, 4, H], F16, name="wcsb")
        for g in range(4):
            nc.sync.dma_start(out=wc_sb[:, g, :],
                              in_=wc[g * 128:(g + 1) * 128, :])
        bqk_sb = cpool.tile([128, 8], F32, name="bqksb")
        nc.sync.dma_start(out=bqk_sb[:, :], in_=bqk[:, :])
        padb_sb = cpool.tile([128, kt], F32, name="padbsb")
        nc.sync.dma_start(out=padb_sb[:, :], in_=padb[:, :])
        eye_sb = cpool.tile([128, 128], F16, name="eyesb")
        nc.sync.dma_start(out=eye_sb[:, :], in_=eyebar[:, :])
        inv_sb = cpool.tile([128, kt], F32, name="invsb")
        nc.sync.dma_start(out=inv_sb[:, :], in_=inv16[:, :])
        ones_sb = cpool.tile([128, 64], F16, name="onessb")
        nc.sync.dma_start(out=ones_sb[:, :], in_=ones64[:, :])

        xc_sb = wpool.tile([128, FT, cap], F16, name="xcsb")
        for ft in range(FT):
            nc.sync.dma_start(out=xc_sb[:, ft, :],
                              in_=xcT[ft * 128:(ft + 1) * 128, :])

        # ---- q/k projections -> qkT [128, 4, cap] f16 ----
        qkT = {w: wpool.tile([128, 4, cap], F16, name=f"{w}T")
               for w in "qk"}
        for wi, w in enumerate("qk"):
            for g in range(4):
                for qo, qn in qch:
                    pp = pspool.tile([128, 512], F32, tag="mm", bufs=2)
                    for ft in range(FT):
                        nc.tensor.matmul(
                            pp[:, 0:qn],
                            w_sb[w][:, ft, g * 128:(g + 1) * 128],
                            xc_sb[:, ft, qo:qo + qn],
                            start=(ft == 0), stop=(ft == FT - 1))
                    nc.vector.tensor_scalar_add(
                        qkT[w][:, g, qo:qo + qn], pp[:, 0:qn],
                        bqk_sb[:, 4 * wi + g:4 * wi + g + 1])

        # ---- v projection (token-major) -> vp [128, kt, 8, 65] f16 ----
        # vp[key, kt, h, m] = V[key, h*64+m]; vp[key, kt, h, 64] = 1 (denom)
        vp = wpool.tile([128, kt, HPC, 65], F16, name="vp")
        for tt in range(kt):
            pv = pspool.tile([128, 512], F32, tag="mm", bufs=2)
            for ft in range(FT):
                nc.tensor.matmul(
                    pv[:, :],
                    xc_sb[:, ft, tt * 128:(tt + 1) * 128],
                    w_sb["v"][:, ft, :],
                    start=(ft == 0), stop=(ft == FT - 1))
            # strided copy with padding-key mask: [128, 8, 64] from pv
            nc.vector.tensor_scalar_mul(
                vp[:, tt, :, 0:64],
                pv.rearrange("p (h d) -> p h d", d=64),
                inv_sb[:, tt:tt + 1])
            nc.vector.tensor_copy(
                vp[:, tt, :, 64:65],
                inv_sb[:, tt:tt + 1].to_broadcast((128, HPC, 1)))

        # ---- attention, 3-stage software pipeline over (pair, qchunk) ----
        # stage 1: dots + exp + diag -> ptp tiles
        # stage 2 (one step behind): AV accumulation + reciprocal issue
        # stage 3 (two steps behind): rep broadcast + normalize
        # This keeps the PE fed: exp(i) overlaps AV(i-1) on the PE, and the
        # slow [1,N] reciprocal gets a full step of slack before rep reads it.
        onorm = wpool.tile([128, 4, cap], F16, name="onorm")
        avq = []        # (ptp pair, p, qo, qn) awaiting AV
        deferred = []   # (osb, rcref, row, p, rb, qo, qn) awaiting normalize
        den_st = {"tile": None, "row": 0, "rc": None}

        def flush_norm(count):
            for _ in range(count):
                if not deferred or deferred[0][1]["rc"] is None:
                    return
                osb_d, rcref, row, p_d, rb_d, qo_d, qn_d = deferred.pop(0)
                rc16 = rcref["rc"]
                rep = pspool.tile([64, 512], F32, tag="mm", bufs=2,
                                  name="rep")
                nc.tensor.matmul(rep[:, 0:qn_d],
                                 ones_sb[32 * row:32 * row + 1, :],
                                 rc16[32 * row:32 * row + 1, 0:qn_d],
                                 start=True, stop=True,
                                 tile_position=(32 * row, 0))
                nc.vector.tensor_mul(
                    onorm[rb_d:rb_d + 64, p_d, qo_d:qo_d + qn_d],
                    osb_d[:, 0:qn_d], rep[:, 0:qn_d])

        def finish_av(avs2, p_a, qo_a, qn_a):
            # drain avs to SBUF (frees PSUM), batch reciprocals
            for h01 in range(2):
                avs = avs2[h01]
                rb = h01 * 64
                osb = spool.tile([64, 512], F16, tag="osb", bufs=6,
                                 name="oun")
                nc.vector.tensor_copy(osb[:, 0:qn_a], avs[0:64, 0:qn_a])
                if den_st["tile"] is None:
                    den_st["tile"] = spool.tile([128, 512], F32,
                                                tag="den4", bufs=2,
                                                name="den4")
                    nc.vector.memset(den_st["tile"][:, :], 1.0)
                    den_st["row"] = 0
                    den_st["rc"] = {"rc": None}
                d4, row = den_st["tile"], den_st["row"]
                nc.vector.tensor_copy(d4[32 * row:32 * row + 1, 0:qn_a],
                                      avs[64:65, 0:qn_a])
                deferred.append((osb, den_st["rc"], row, p_a, rb,
                                 qo_a, qn_a))
                den_st["row"] += 1
                if den_st["row"] == 4:
                    # one reciprocal covers 4 denominators (4 DVE lanes
                    # run in parallel; [1,N] and [4,N] cost the same)
                    rc32 = spool.tile([128, 512], F32, tag="rc32",
                                      bufs=2, name="rc32")
                    with nc.allow_low_precision(
                            reason="1/den in fp16: 0.05% rel, den>=13"):
                        nc.vector.reciprocal(rc32[0:97, :], d4[0:97, :])
                    rc16 = spool.tile([128, 512], F16, tag="rc16", bufs=2,
                                      name="rc16")
                    nc.vector.tensor_copy(rc16[0:97, :], rc32[0:97, :])
                    den_st["rc"]["rc"] = rc16
                    den_st["tile"] = None

        for p in range(4):
            for qo, qn in qch:
                flush_norm(2)
                prev = avq.pop(0) if avq else None
                if prev:
                    ptp_a, p_a, qo_a, qn_a = prev
                    avs2 = [pspool.tile([65, 512], F32, tag="avs", bufs=2,
                                        name="avs") for _ in range(2)]
                ptp = ppool.tile([128, kt, 2, 512], F16, tag="ptp",
                                 bufs=2, name=f"ptp{p}{qo}")
                for k in range(kt):
                    # both heads' dots adjacent at row groups 0/64 -> the
                    # PE runs them concurrently; one exp covers both.
                    # Previous step's AV matmuls are interleaved per-k so
                    # the PE has ready work while exp paces the dots.
                    dp = pspool.tile([128, 2, 512], F32, tag="dp", bufs=2)
                    for h01 in range(2):
                        rb = h01 * 64
                        nc.tensor.matmul(
                            dp[:, h01, 0:qn],
                            qkT["k"][rb:rb + 64, p,
                                     k * 128:(k + 1) * 128],
                            qkT["q"][rb:rb + 64, p, qo:qo + qn],
                            start=True, stop=True,
                            tile_position=(rb, 0))
                    nc.scalar.activation(
                        ptp[:, k, :, 0:qn], dp[:, :, 0:qn],
                        EXP, scale=0.125, bias=padb_sb[:, 0:1])
                    # diagonal zeroing when this kt's window is in-chunk
                    if qo <= k * 128 < qo + qn:
                        off = k * 128 - qo
                        w = min(qn - off, 128)
                        for h01 in range(2):
                            nc.vector.tensor_mul(
                                ptp[:, k, h01, off:off + w],
                                ptp[:, k, h01, off:off + w],
                                eye_sb[:, 0:w])
                    if prev:
                        for h01 in range(2):
                            nc.tensor.matmul(
                                avs2[h01][:, 0:qn_a],
                                vp[:, k, 2 * p_a + h01, :],
                                ptp_a[:, k, h01, 0:qn_a],
                                start=(k == 0), stop=(k == kt - 1),
                                perf_mode=DR)
                if prev:
                    finish_av(avs2, p_a, qo_a, qn_a)
                avq.append((ptp, p, qo, qn))
        # tail: AV + normalize for the final step
        ptp_a, p_a, qo_a, qn_a = avq.pop(0)
        avs2 = [pspool.tile([65, 512], F32, tag="avs", bufs=2, name="avs")
                for _ in range(2)]
        for k in range(kt):
            for h01 in range(2):
                nc.tensor.matmul(
                    avs2[h01][:, 0:qn_a],
                    vp[:, k, 2 * p_a + h01, :],
                    ptp_a[:, k, h01, 0:qn_a],
                    start=(k == 0), stop=(k == kt - 1),
                    perf_mode=DR)
        finish_av(avs2, p_a, qo_a, qn_a)
        flush_norm(len(deferred))

        # ---- output projection ----
        for tt in range(kt):
            osb = opool.tile([128, H], F16, tag="osb")
            for oc in range(2):
                op = pspool.tile([128, 512], F32, tag="mm", bufs=2)
                for g in range(4):
                    nc.tensor.matmul(
                        op[:, :],
                        onorm[:, g, tt * 128:(tt + 1) * 128],
                        wc_sb[:, g, oc * 512:(oc + 1) * 512],
                        start=(g == 0), stop=(g == 3))
                nc.vector.tensor_copy(osb[:, oc * 512:(oc + 1) * 512],
                                      op[:, :])
            nc.sync.dma_start(
                out=outp[tt * 128:(tt + 1) * 128, :], in_=osb[:, :])
    nc.finalize()
    return nc


_NC_CACHE = {}


def _get_nc(cap, fused_pairs):
    key = (cap, fused_pairs)
    if key not in _NC_CACHE:
        _NC_CACHE[key] = build_bass(cap, fused_pairs)
    return _NC_CACHE[key]


def kernel(encoder_outputs, mask, Wq, bq, Wk, bk, Wv, bv, Wc):
    global LAST_RESULTS
    x = np.asarray(encoder_outputs, dtype=np.float32)
    mask = np.asarray(mask)
    f16 = np.float16
    Wqh, Wkh, Wvh = (np.asarray(w, np.float32) for w in (Wq, Wk, Wv))
    Wch = np.asarray(Wc, np.float32)

    idxs = [np.where(mask[b] == 0)[0] for b in range(B)]
    nbs = [len(i) for i in idxs]
    if max(nbs) <= 1024 + 64 and min(nbs) >= 896:
        cap, fused = CAP, 3    # attention q-range trimmed to 1088
    else:
        cap, fused = S, 0          # generic fallback: no compaction gain
    kt = cap // 128

    eyebar = (1.0 - np.eye(128, dtype=np.float32)).astype(f16)
    onesv = np.ones((128, 64), dtype=f16)

    in_maps = []
    for c in range(NCORES):
        b, half = c // 2, c % 2
        hsl = slice(half * PD, (half + 1) * PD)
        idx, nb = idxs[b], nbs[b]
        xc = np.zeros((cap, H), np.float32)
        xc[:nb] = x[b, idx[:nb]]
        xcT = np.ascontiguousarray(xc.T).astype(f16)
        padb = np.full(cap, -1.0, dtype=np.float32)
        inv = (np.arange(cap) < nb).astype(np.float32)
        bqk = np.stack([np.asarray(bq, np.float32)[hsl].reshape(4, 128),
                        np.asarray(bk, np.float32)[hsl].reshape(4, 128)]
                       ).reshape(8, 128).T.copy()
        in_maps.append({
            "xcT": xcT,
            "wq": np.ascontiguousarray(Wqh[hsl, :].T).astype(f16),
            "wk": np.ascontiguousarray(Wkh[hsl, :].T).astype(f16),
            "wv": np.ascontiguousarray(Wvh[hsl, :].T).astype(f16),
            "wc": np.ascontiguousarray(Wch[:, hsl].T).astype(f16),
            "bqk": bqk,
            "padb": np.ascontiguousarray(padb.reshape(kt, 128).T),
            "eyebar": eyebar,
            "inv16": np.ascontiguousarray(
                inv.reshape(kt, 128).T.astype(np.float32)),
            "ones64": onesv,
        })

    res = run_bass_kernel_spmd(
        _get_nc(cap, fused), in_maps, list(range(NCORES)),
        trace=bool(os.environ.get("BASS_TRACE")))
    LAST_RESULTS = res

    bvWc = (np.asarray(bv, np.float64) @ np.asarray(Wc, np.float64).T
            ).astype(np.float32)
    out = np.zeros((B, S, H), dtype=np.float32)
    for b in range(B):
        nb = nbs[b]
        part = (res.results[2 * b]["out"][:nb].astype(np.float32)
                + res.results[2 * b + 1]["out"][:nb].astype(np.float32))
        out[b, idxs[b][:nb]] = part + bvWc[None, :]
    return out
